# revision 18
# baseline (speedup 1.0000x reference)
"""KoLeo loss kernel for Trainium2 (8 NeuronCores, Bass/Tile) — symmetric Gram.

reference semantics:
    x = student_output / max(||row||_2, 1e-8)        # [B, D] row-normalize
    dots = x @ x.T ; dots[i,i] = -1
    nn = argmax(dots, axis=1)
    d_i = || x_i - x_nn(i) + 1e-8 ||_2
    loss = mean(-log(d_i + 1e-8))

Device strategy (symmetric Gram, 8 cores, identical NEFF):
  * dots is symmetric, so block (i,j) and (j,i) need not both be computed.
    Core p computes blocks (p, p+d mod 8), d = 0..4, on fully NORMALIZED
    bf16 operands (so PSUM holds true cosine dots, no drain-scaling):
      - d=0 (diagonal block): tiles (mt 0-3, strip0) + (mt 0-7, strip1);
        the dropped lower-left tiles are recovered from the column side of
        the (mt 0-3, strip1) tiles by symmetry.
      - d=1..3: all 16 [128x512] tile-groups.
      - d=4: the +4 pair is shared with core p+4: quadrants Q00 (mt 0-3,
        strip0), Q01 (mt 0-3, strip1), Q11 (mt 4-7, strip1). Q00/Q11 are
        double-computed globally (harmless under max), Q10 comes from the
        partner's Q01 column side.
    => 72 tile-groups x 8 K-matmuls = 576 MMs/core (vs 1024 baseline).
  * Normalization: per column strip, ACT Square (per k-chunk) -> DVE
    tree-adds (bf16) -> ones-matmul (partition sum) -> ACT rsqrt -> bf16
    rb -> 8 DVE muls produce xn = x/||x|| in bf16.
  * Row-side candidates: DVE max8 straight from PSUM -> cand[P,mt,slot,8].
  * Column-side candidates (for rows owned by other cores / own lower
    half): DVE tensor_max chains over mt into colacc[P,512], DMA'd out.
  * Host combines: per row 2nd-max of own candidate pool (self-dot ~= 1.0
    is the max) max'd with column-side contributions from the 4 source
    cores, then loss = mean(-0.5*ln(2-2m)).  Host-side cost is microseconds
    of numpy on [8192] vectors.
"""

import numpy as np
import ml_dtypes

import concourse.bacc as bacc
import concourse.bass as bass
import concourse.mybir as mybir
import concourse.tile as tile
from concourse import bass_utils

B, D, P = 8192, 1024, 128
NCORES = 8
LOCAL = B // NCORES  # 1024 rows per core
KT = D // P          # 8 contraction tiles
MT = LOCAL // P      # 8 local row tiles
NJ = 512             # moving free dim per matmul
NBLK = 5             # blocks p..p+4 held per core
NSLOT = 10           # cand slots per (row, mt): (d,s) pairs
NCOL = 9             # colacc strips: d0s1, d1s0..d4s1
WARM_MM = 24         # PE warmups during DMA prologue (single accumulation group)

F32 = mybir.dt.float32
BF16 = mybir.dt.bfloat16
FP8 = mybir.dt.float8e4
AF = mybir.ActivationFunctionType
USE_FP8 = True       # fp8e4 normalized operands + DoubleRow matmuls (2x PE)
XDT = FP8 if USE_FP8 else BF16
KS = 2 if USE_FP8 else 1     # contraction subtiles per matmul
PERF = mybir.MatmulPerfMode.DoubleRow if USE_FP8 else None


def mt_range(d, s):
    """Row tiles computed for stage d, strip s."""
    if d in (0, 4) and s == 0:
        return range(4)
    return range(MT)


def col_chain(d, s):
    """mt's contributing to the column-side accumulator for (d, s).
    None => no column-side extraction for this strip."""
    if d == 0:
        return range(4) if s == 1 else None
    if d == 4 and s == 0:
        return range(4)
    return range(MT)


def emit_kernel(tc, x_ap, cand_ap, colmax_ap):
    nc = tc.nc
    with (
        tc.tile_pool(name="big", bufs=1) as big,
        tc.tile_pool(name="raw", bufs=3) as rawp,
        tc.tile_pool(name="xn", bufs=2) as xnp,
        tc.tile_pool(name="sq", bufs=2) as sqp,
        tc.tile_pool(name="work", bufs=3) as work,
        tc.tile_pool(name="ca", bufs=4) as cap_,
        tc.tile_pool(name="ps", bufs=5, space="PSUM") as pp,
        tc.tile_pool(name="ps2", bufs=2, space="PSUM") as pp2,
        tc.tile_pool(name="psw", bufs=1, space="PSUM") as ppw,
    ):
        ones = big.tile([P, P], BF16)
        nc.vector.memset(ones[:], 1.0)
        gwarm = big.tile([P, NJ], BF16)
        nc.vector.memset(gwarm[:], 0.5)
        cand = big.tile([P, MT, NSLOT, 8], F32)
        nc.vector.memset(cand[:], -2.0)
        xnl = big.tile([P, KT, LOCAL], XDT)  # normalized local block

        # warm the ACT function tables before they gate the pipeline
        warm = big.tile([P, 1], F32)
        nc.scalar.activation(warm[:], ones[:, :1], AF.Square)
        nc.scalar.activation(warm[:], ones[:, :1], AF.Abs_reciprocal_sqrt)

        # PE warmups: one long accumulation group (back-to-back MMs, no
        # per-MM pool-slot serialization) to hold the HAM activity window
        # open while the prologue DMAs + norm pipeline run.
        pw = ppw.tile([P, NJ], F32, tag="warm")
        for w in range(WARM_MM):
            nc.tensor.matmul(
                pw[:], ones[:], gwarm[:], start=(w == 0), stop=(w == WARM_MM - 1)
            )

        # ---- DMA: block d lands in a ring slot, per-k chunks ----
        def dma_block(d):
            raw = rawp.tile([P, KT, LOCAL], BF16, tag="raw")
            for k in range(KT):
                nc.sync.dma_start(
                    out=raw[:, k], in_=x_ap[k, :, d * LOCAL : (d + 1) * LOCAL]
                )
            return raw

        # ---- norms + normalize for one block ----
        def norm_block(d, raw):
            xn = xnl if d == 0 else xnp.tile([P, KT, LOCAL], XDT, tag="xn")
            sq = sqp.tile([P, KT, LOCAL], BF16, tag="sq")
            for k in range(KT):
                nc.scalar.activation(sq[:, k], raw[:, k], AF.Square)
            for s in (0, 1):
                jb = slice(s * NJ, (s + 1) * NJ)
                a = work.tile([P, 4, NJ], BF16, tag="tra")
                nc.vector.tensor_add(a[:], sq[:, 0:4, jb], sq[:, 4:8, jb])
                b2 = work.tile([P, 2, NJ], BF16, tag="trb")
                nc.vector.tensor_add(b2[:], a[:, 0:2], a[:, 2:4])
                c = work.tile([P, NJ], BF16, tag="trc")
                nc.vector.tensor_add(c[:], b2[:, 0], b2[:, 1])
                psn = pp2.tile([P, NJ], F32, tag="psn")
                nc.tensor.matmul(psn[:], ones[:], c[:], start=True, stop=True)
                rb = work.tile([P, NJ], BF16, tag="rb")
                nc.scalar.activation(rb[:], psn[:], AF.Abs_reciprocal_sqrt)
                for k in range(KT):
                    nc.vector.tensor_mul(xn[:, k, jb], raw[:, k, jb], rb[:])
            return xn

        # ---- one Gram strip: matmuls + row max8 + column-side chain ----
        def gram_strip(d, s, xn):
            jb = slice(s * NJ, (s + 1) * NJ)
            slot = d * 2 + s
            chain = col_chain(d, s)
            ca = None
            if chain is not None:
                ca = cap_.tile([P, NJ], F32, tag="ca")
                chain = set(chain)
            for mt in mt_range(d, s):
                ps = pp.tile([P, NJ], F32, tag="ps_u")
                for t in range(KT // KS):
                    kk = slice(t * KS, (t + 1) * KS)
                    nc.tensor.matmul(
                        ps[:],
                        xnl[:, kk, mt * P : (mt + 1) * P],
                        xn[:, kk, jb],
                        start=(t == 0),
                        stop=(t == KT // KS - 1),
                        perf_mode=PERF,
                    )
                nc.vector.max(out=cand[:, mt, slot], in_=ps[:])
                if chain is not None and mt in chain:
                    if mt == min(chain):
                        nc.vector.tensor_copy(ca[:], ps[:])
                    else:
                        nc.vector.tensor_max(ca[:], ps[:], ca[:])
            return ca

        # colmax strip order: d0s1, then (d,s) for d=1..4
        def col_idx(d, s):
            if d == 0:
                return 0
            return 1 + (d - 1) * 2 + s

        # ---- main pipeline ----
        raws = {0: dma_block(0), 1: dma_block(1)}
        xns = {0: norm_block(0, raws.pop(0))}

        for d in range(NBLK):
            xn = xns.pop(d)
            for s in (0, 1):
                ca = gram_strip(d, s, xn)
                if ca is not None:
                    i = col_idx(d, s)
                    nc.sync.dma_start(
                        out=colmax_ap[:, i * NJ : (i + 1) * NJ], in_=ca[:]
                    )
                # interleave next block's prep between the two strips
                if s == 0 and d + 1 < NBLK:
                    if d + 2 < NBLK and (d + 2) not in raws:
                        raws[d + 2] = dma_block(d + 2)
                    xns[d + 1] = norm_block(d + 1, raws.pop(d + 1))

        nc.sync.dma_start(out=cand_ap, in_=cand[:])


def build_bass():
    nc = bacc.Bacc(
        "TRN2",
        target_bir_lowering=False,
        debug=False,
        enable_asserts=True,
        num_devices=NCORES,
    )
    x_t = nc.dram_tensor("xbf", [KT, P, NBLK * LOCAL], BF16, kind="ExternalInput").ap()
    cand_t = nc.dram_tensor(
        "cand", [P, MT * NSLOT * 8], F32, kind="ExternalOutput"
    ).ap()
    colmax_t = nc.dram_tensor(
        "colmax", [P, NCOL * NJ], F32, kind="ExternalOutput"
    ).ap()
    with tile.TileContext(nc) as tc:
        emit_kernel(tc, x_t, cand_t, colmax_t)
    nc.compile()
    return nc


def make_in_maps(x: np.ndarray):
    xbf = x.astype(ml_dtypes.bfloat16)
    # [KT, P, B]: element [k, p, r] = x[r, k*128 + p]  (transposed layout)
    xt = np.ascontiguousarray(xbf.reshape(B, KT, P).transpose(1, 2, 0))
    maps = []
    for c in range(NCORES):
        cols = [
            xt[:, :, ((c + d) % NCORES) * LOCAL : ((c + d) % NCORES + 1) * LOCAL]
            for d in range(NBLK)
        ]
        maps.append({"xbf": np.ascontiguousarray(np.concatenate(cols, axis=2))})
    return maps


def reduce_outputs(results):
    row2nd = np.empty((NCORES, LOCAL), np.float64)
    contrib = np.empty((NCORES, 4, LOCAL), np.float64)
    c0 = np.empty((NCORES, NJ), np.float64)
    for p, r in enumerate(results):
        cand = np.asarray(r["cand"], dtype=np.float64).reshape(P, MT, NSLOT * 8)
        pool = cand.transpose(1, 0, 2).reshape(LOCAL, NSLOT * 8)
        row2nd[p] = np.partition(pool, -2, axis=1)[:, -2]
        cm = np.asarray(r["colmax"]).astype(np.float64).reshape(P, NCOL, NJ).max(axis=0)
        c0[p] = cm[0]
        contrib[p] = cm[1:].reshape(4, LOCAL)
    m = row2nd.copy()
    for b in range(NCORES):
        m[b, NJ:] = np.maximum(m[b, NJ:], c0[b])
        for d in range(1, NBLK):
            src = (b - d) % NCORES
            m[b] = np.maximum(m[b], contrib[src, d - 1])
    d2 = 2.0 - 2.0 * m
    losses = -0.5 * np.log(d2)
    return np.array(losses.mean(), dtype=np.float32)


_LAST_RESULTS = None  # BassKernelResults of the most recent run (for test.py)


def run(x: np.ndarray, trace: bool = False):
    global _LAST_RESULTS
    nc = build_bass()
    res = bass_utils.run_bass_kernel_spmd(
        nc,
        make_in_maps(x),
        core_ids=list(range(NCORES)),
        trace=trace,
        trace_cores=list(range(NCORES)) if trace else None,
    )
    _LAST_RESULTS = res
    return reduce_outputs(res.results)


def kernel(**inputs) -> np.ndarray:
    x = np.asarray(inputs["student_output"], dtype=np.float32)
    assert x.shape == (B, D), x.shape
    return run(x, trace=False)


if __name__ == "__main__":
    rng = np.random.default_rng(0)
    x = rng.standard_normal((B, D), dtype=np.float32)
    print(kernel(student_output=x))
